# revision 75
# baseline (speedup 1.0000x reference)
"""Causal multi-head attention on 8 Trainium2 NeuronCores.

Problem: B=2, S=2048, H=1024, NH=16, HD=64, fp32. Tensor-parallel over
heads (2 heads/core) + AllToAll of attention context so every core runs
the output projection for its own 512-token slice.

v2 design notes (vs v1 baseline, 251 us -> 181 us):
- All transposed layouts (x^T tiled per chunk, Wq/Wk/Wv^T slices, Wo^T)
  are prepared on the HOST in the exact SBUF layout and pre-cast to bf16:
  one large DMA per tensor/chunk, no PE transposes, no staging copies.
- All matmul data bf16 (walrus requires matching operand dtypes when
  either is f32/f32r): 1 col/cycle at any width (fp32r pays 4x under 256
  cols), transposes at 1.0 cyc/row, A2A payloads halved (28.1us each).
- QKV biases folded into the PSUM->SBUF copies as per-partition
  tensor_scalar adds on DVE (GPSIMD cannot touch PSUM on real HW).
- ACT runs ONLY the exp stream; S^T for two k-tiles lands in one 2-bank
  PSUM tile so a single exp covers both (halves ACT instruction count).
- PE pstate ramp (cost model: 2x for 3us after any idle) is kept alive
  with warmup matmuls on resident data across the startup and X1 windows.
- Output projection split by head half: the head-0 half (depends only on
  X0) runs as real work inside the X1 window; post-X1 only the head-1
  half plus a DVE add remain.

Schedule per core c (heads 2c, 2c+1 = channels 128c..128c+127):
  L1. Per 512-token chunk: DMA x^T, project q/k (bias-add copy to bf16
      SBUF), project v, PE-transpose v into V1 = [V_h0 | 1 | V_h1 | 1],
      head-0 attention (S^T = K^T.T @ Q^T with diagonal k-tiles narrowed,
      P = exp(S^T/8) -> bf16, masked on the diagonal; ctx[65, 512] +=
      V1.T @ P with row 64 the softmax denominator; DVE reciprocal +
      GPSIMD partition broadcast normalize). The last pair's ctx matmuls
      and the normalize are deferred past the next chunk's QKV.
  X0. AllToAll of head-0 ctx (overlaps L2).
  L2. Head-1 attention as one flattened pair stream, cost-interleaved
      chunk order (matmul-based denominator broadcast; GPSIMD stays
      clear of the active collective).
  X1. AllToAll of head-1 ctx; E_h0 + warmups fill the window.
  E.  out = ctx^T @ Wo^T + bo per head half, DVE add, DMA; host concat.
"""
import sys

if '/opt/trn_rl_repo' not in sys.path:
    sys.path.insert(0, '/opt/trn_rl_repo')

import numpy as np

import concourse.bacc as bacc
import concourse.bass as bass
import concourse.mybir as mybir
from concourse.tile import TileContext
from concourse.bass_utils import run_bass_kernel_spmd
from concourse.masks import make_identity, make_upper_triangular

F32 = mybir.dt.float32
F32R = mybir.dt.float32r
BF16 = mybir.dt.bfloat16
EXP = mybir.ActivationFunctionType.Exp

B, S, H, NH, HD = 2, 2048, 1024, 16, 64
NC = 8
T = B * S                 # 4096 tokens
TC = 512                  # tokens per chunk
NCHUNK = T // TC          # 8
NTT = T // 128            # 32 token tiles
HT = H // 128             # 8 H-tiles
SCALE = 1.0 / np.sqrt(HD)

_cache = {}

AHEAD = 3                 # S-matmul lookahead (st PSUM bufs = AHEAD + 1)


def _l2_pass(nc, pc, qpool, qT, kT, v1, ones_b, ut, a2a_in, order, h,
             baked=None):
    """Head-h attention for all chunks as one flattened pair stream: the
    2-deep S-matmul prefetch runs across chunk boundaries so the st ring
    never drains at a chunk edge. Normalization uses the matmul broadcast
    (GPSIMD must stay clear of the active X0 collective).

    baked: {ch: p_ap} of chunks whose exp(S^T) was precomputed during the
    PE-bound L1 pass (L2 is ACT-bound, so those exps are free there);
    only their ctx matmuls run here."""
    baked = baked or {}
    items = []
    for ch in order:
        npair = (4 * (ch % 4) + 4) // 2
        for kp in range(npair):
            items.append((ch, kp, npair))
    sts = {}

    def emit_pair(idx):
        ch, kp, _ = items[idx]
        b, lc = ch // 4, ch % 4
        stp = qpool.tile([128, 1024], F32, tag='st', bufs=2, name='st')
        for j in range(2):
            kt = 2 * kp + j
            g = 16 * b + kt
            s = kt - 4 * lc
            c0 = 128 * s if s >= 0 else 0
            nc.tensor.matmul(
                stp[:, 512 * j + c0:512 * (j + 1)],
                kT[64 * h:64 * (h + 1), 128 * g:128 * (g + 1)],
                qT[64 * h:64 * (h + 1), TC * ch + c0:TC * (ch + 1)],
                start=True, stop=True)
        sts[idx] = stp

    # 2-deep prefetch over LIVE (non-baked) pairs only, so the st ring
    # stays full across baked chunks
    live = [i for i, (ch, _, _) in enumerate(items) if ch not in baked]
    emitted = [0]

    def emit_next():
        if emitted[0] < len(live):
            emit_pair(live[emitted[0]])
            emitted[0] += 1

    emit_next()
    emit_next()
    ctx_ps = None
    for idx, (ch, kp, npair) in enumerate(items):
        b, lc = ch // 4, ch % 4
        nkt = 2 * npair
        if kp == 0:
            ctx_ps = qpool.tile([128, 512], F32, tag='ctx', bufs=2, name='ctx')

        def col0(kt):
            s = kt - 4 * lc
            return 128 * s if s >= 0 else 0

        c00, c01 = col0(2 * kp), col0(2 * kp + 1)
        if ch in baked:
            p = baked[ch][:, 1024 * kp:1024 * (kp + 1)]
        else:
            stp = sts.pop(idx)
            p = pc.tile([128, 1024], BF16, tag='p', bufs=3, name='p')
            if c01 < 384:
                nc.scalar.activation(p[:, c00:1024], stp[:, c00:1024], EXP,
                                     scale=float(SCALE))
            else:
                nc.scalar.activation(p[:, c00:512], stp[:, c00:512], EXP,
                                     scale=float(SCALE))
                nc.scalar.activation(p[:, 512 + c01:1024],
                                     stp[:, 512 + c01:1024], EXP,
                                     scale=float(SCALE))
            for j, c0 in ((0, c00), (1, c01)):
                if 2 * kp + j - 4 * lc >= 0:
                    nc.vector.tensor_mul(
                        p[:, 512 * j + c0:512 * j + c0 + 128],
                        p[:, 512 * j + c0:512 * j + c0 + 128], ut[:])
            emit_next()
        for j, c0 in ((0, c00), (1, c01)):
            kt = 2 * kp + j
            g = 16 * b + kt
            nc.tensor.matmul(
                ctx_ps[0:65, c0:512],
                v1[:, 130 * g + 65 * h:130 * g + 65 * h + 65],
                p[:, 512 * j + c0:512 * (j + 1)],
                start=(kt == 0), stop=(kt == nkt - 1))
        if kp == npair - 1:
            recip_f = pc.tile([1, 512], F32, tag='recip_f', bufs=2,
                              name='recip_f')
            nc.vector.reciprocal(recip_f[:], ctx_ps[64:65, :])
            recip_b = pc.tile([1, 512], BF16, tag='recip_b', bufs=2,
                              name='recip_b')
            nc.vector.tensor_copy(recip_b[:], recip_f[:])
            bc = qpool.tile([128, 512], F32, tag='work', bufs=2, name='bc')
            nc.tensor.matmul(bc[0:64, :], ones_b[0:1, 0:64], recip_b[:],
                             start=True, stop=True)
            bc_sb = pc.tile([64, 512], F32, tag='bc_sb', bufs=2, name='bc_sb')
            nc.vector.tensor_copy(bc_sb[:], bc[0:64, :])
            ctx_sb = pc.tile([64, 512], BF16, tag='ctx_sb', bufs=3,
                             name='ctx_sb')
            nc.vector.tensor_mul(ctx_sb[:], ctx_ps[0:64, :], bc_sb[:])
            nc.sync.dma_start(a2a_in[ch, :, :], ctx_sb[:])


def _attention(nc, pc, qpool, qT, kT, v1, ones_b, ut, a2a_in, ch, h,
               use_pb=True, defer_finish=False):
    """Head-h causal attention for token chunk ch; writes ctx to a2a_in.

    S-matmuls are emitted AHEAD iterations early so the PE never waits on
    ACT. V1 blocks are [V_h0 | 1 | V_h1 | 1] (width 130): head h uses cols
    [65h : 65h+65] = (V_h | ones), so ctx lands in rows 0:64 and the softmax
    denominator in row 64.

    With defer_finish=True, the last pair's ctx matmuls + normalization are
    returned as a closure instead of emitted: the caller emits the next
    chunk's QKV first, so those ACT-gated matmuls never block it in the
    in-order PE queue.
    """
    b, lc = ch // 4, ch % 4
    nkt = 4 * lc + 4
    npair = nkt // 2
    ctx_ps = qpool.tile([128, 512], F32, tag='ctx', bufs=2, name='ctx')

    def col0(kt):
        s = kt - 4 * lc
        return 128 * s if s >= 0 else 0

    sts = {}

    def emit_s(kp):
        # S^T for k-tiles (2kp, 2kp+1) land in one 2-bank PSUM tile so a
        # single exp instruction covers both (halves ACT instruction count).
        stp = qpool.tile([128, 1024], F32, tag='st', bufs=2, name='st')
        for j in range(2):
            kt = 2 * kp + j
            g = 16 * b + kt
            c0 = col0(kt)
            nc.tensor.matmul(
                stp[:, 512 * j + c0:512 * (j + 1)],
                kT[64 * h:64 * (h + 1), 128 * g:128 * (g + 1)],
                qT[64 * h:64 * (h + 1), TC * ch + c0:TC * (ch + 1)],
                start=True, stop=True)
        sts[kp] = stp

    def emit_ctx(kp, p, c00, c01):
        for j, c0 in ((0, c00), (1, c01)):
            kt = 2 * kp + j
            g = 16 * b + kt
            nc.tensor.matmul(
                ctx_ps[0:65, c0:512],
                v1[:, 130 * g + 65 * h:130 * g + 65 * h + 65],
                p[:, 512 * j + c0:512 * (j + 1)],
                start=(kt == 0), stop=(kt == nkt - 1))

    for j in range(min(2, npair)):
        emit_s(j)
    last = None
    for kp in range(npair):
        c00, c01 = col0(2 * kp), col0(2 * kp + 1)
        stp = sts.pop(kp)
        p = pc.tile([128, 1024], BF16, tag='p', bufs=3, name='p')
        if c01 < 384:
            # [512 : 512+c01) was never written; exp of stale PSUM there is
            # finite garbage that no ctx matmul reads.
            nc.scalar.activation(p[:, c00:1024], stp[:, c00:1024], EXP,
                                 scale=float(SCALE))
        else:
            # gap too wide: two exps cost less than exp-ing the dead span
            nc.scalar.activation(p[:, c00:512], stp[:, c00:512], EXP,
                                 scale=float(SCALE))
            nc.scalar.activation(p[:, 512 + c01:1024], stp[:, 512 + c01:1024],
                                 EXP, scale=float(SCALE))
        for j, c0 in ((0, c00), (1, c01)):
            if 2 * kp + j - 4 * lc >= 0:
                nc.vector.tensor_mul(p[:, 512 * j + c0:512 * j + c0 + 128],
                                     p[:, 512 * j + c0:512 * j + c0 + 128],
                                     ut[:])
        if kp + 2 < npair:
            emit_s(kp + 2)
        if defer_finish and kp == npair - 1:
            last = (kp, p, c00, c01)
        else:
            emit_ctx(kp, p, c00, c01)

    def finish():
        if last is not None:
            emit_ctx(*last)
        _normalize(nc, pc, qpool, ctx_ps, ones_b, a2a_in, ch, use_pb)

    if defer_finish:
        return finish
    finish()


def _normalize(nc, pc, qpool, ctx_ps, ones_b, a2a_in, ch, use_pb):
    recip_f = pc.tile([1, 512], F32, tag='recip_f', bufs=2, name='recip_f')
    nc.vector.reciprocal(recip_f[:], ctx_ps[64:65, :])
    if use_pb:
        # GPSIMD broadcast — only safe while no collective occupies Pool
        bc_sb = pc.tile([64, 512], F32, tag='bc_sb', bufs=2, name='bc_sb')
        nc.gpsimd.partition_broadcast(bc_sb[:], recip_f[:])
    else:
        recip_b = pc.tile([1, 512], BF16, tag='recip_b', bufs=2, name='recip_b')
        nc.vector.tensor_copy(recip_b[:], recip_f[:])
        bc = qpool.tile([128, 512], F32, tag='work', bufs=2, name='bc')
        nc.tensor.matmul(bc[0:64, :], ones_b[0:1, 0:64], recip_b[:],
                         start=True, stop=True)
        bc_sb = pc.tile([64, 512], F32, tag='bc_sb', bufs=2, name='bc_sb')
        nc.vector.tensor_copy(bc_sb[:], bc[0:64, :])
    ctx_sb = pc.tile([64, 512], BF16, tag='ctx_sb', bufs=3, name='ctx_sb')
    nc.vector.tensor_mul(ctx_sb[:], ctx_ps[0:64, :], bc_sb[:])
    nc.sync.dma_start(a2a_in[ch, :, :], ctx_sb[:])


def _build(phases='LE'):
    key = ('nc', phases)
    if key in _cache:
        return _cache[key]
    nc = bacc.Bacc('TRN2', target_bir_lowering=False, debug=False, num_devices=NC)

    # Host pre-tiles everything into the exact SBUF layout: one DMA per
    # tensor (chunk), 2KB+ contiguous runs per partition.
    hst_d = nc.dram_tensor('hst', [NCHUNK, 128, HT * TC], BF16,
                           kind='ExternalInput')
    wqt_d = nc.dram_tensor('wqt', [128, H], BF16, kind='ExternalInput')
    wkt_d = nc.dram_tensor('wkt', [128, H], BF16, kind='ExternalInput')
    wvt_d = nc.dram_tensor('wvt', [128, H], BF16, kind='ExternalInput')
    wot_d = nc.dram_tensor('wot', [128, H * HT], BF16, kind='ExternalInput')
    bq_d = nc.dram_tensor('bq', [128, 1], F32, kind='ExternalInput')
    bk_d = nc.dram_tensor('bk', [128, 1], F32, kind='ExternalInput')
    bv_d = nc.dram_tensor('bv', [128, 1], F32, kind='ExternalInput')
    bo_d = nc.dram_tensor('bo', [1, H], F32, kind='ExternalInput')
    out_d = nc.dram_tensor('out', [TC, H], F32, kind='ExternalOutput')

    with TileContext(nc) as tc:
        with tc.tile_pool(name='persist', bufs=1) as pp, \
             tc.tile_pool(name='scr', bufs=1) as sc, \
             tc.tile_pool(name='dram', bufs=1, space='DRAM') as dpool, \
             tc.tile_pool(name='psum', bufs=1, space='PSUM') as qpool:

            def ptile(shape, dt, tag):
                return pp.tile(shape, dt, tag=tag, name=tag)

            # ones goes first: a single memset, so PE warmups can start
            # within ~200ns of t=0 (the masks below take ~2us to build)
            ones_f = ptile([1, 128], F32, 'ones_f')
            nc.vector.memset(ones_f[:], 1.0)

            ident_f = ptile([128, 128], F32, 'ident_f')
            make_identity(nc, ident_f[:])
            ident = ptile([128, 128], BF16, 'ident')
            nc.vector.tensor_copy(ident[:], ident_f[:])
            ut_f = ptile([128, 128], F32, 'ut_f')
            make_upper_triangular(nc, ut_f[:], val=1.0, diag=True)
            ut = ptile([128, 128], BF16, 'ut')
            nc.vector.tensor_copy(ut[:], ut_f[:])
            ones_r = ptile([1, 128], F32R, 'ones_r')
            nc.vector.tensor_copy(ones_r[:], ones_f[:])
            ones_b = ptile([1, 128], BF16, 'ones_b')
            nc.vector.tensor_copy(ones_b[:], ones_f[:])

            # First chunk of x goes out before everything else so the first
            # QKV matmuls are never queued behind the weight preamble.
            def load_x(ch):
                xt = sc.tile([128, HT * TC], BF16, tag='xT', bufs=2, name='xT')
                # halves: the next chunk's QKV can start on the first half
                nc.sync.dma_start(xt[:, 0:4 * TC], hst_d[ch, :, 0:4 * TC])
                nc.sync.dma_start(xt[:, 4 * TC:], hst_d[ch, :, 4 * TC:])
                return xt

            # Weights: host-pretransposed, pre-tiled, bf16; one DMA each.
            # Interleaved with the split first x chunk: the first QKV
            # accumulations need x[0:4] + wq, so those transfers go first.
            wqT = ptile([128, H], BF16, 'wqT')
            wkT = ptile([128, H], BF16, 'wkT')
            wvT = ptile([128, H], BF16, 'wvT')
            next_xt = sc.tile([128, HT * TC], BF16, tag='xT', bufs=2,
                              name='xT')
            nc.sync.dma_start(next_xt[:, 0:4 * TC], hst_d[0, :, 0:4 * TC])
            nc.sync.dma_start(wqT[:], wqt_d[:])
            nc.sync.dma_start(next_xt[:, 4 * TC:], hst_d[0, :, 4 * TC:])
            nc.sync.dma_start(wkT[:], wkt_d[:])
            nc.sync.dma_start(wvT[:], wvt_d[:])

            bq_sb = ptile([128, 1], F32, 'bq_sb')
            bk_sb = ptile([128, 1], F32, 'bk_sb')
            bv_sb = ptile([128, 1], F32, 'bv_sb')
            for dst, src in ((bq_sb, bq_d), (bk_sb, bk_d), (bv_sb, bv_d)):
                nc.sync.dma_start(dst[:], src[:])
            bo_f = ptile([1, H], F32, 'bo_f')
            nc.sync.dma_start(bo_f[:], bo_d[:])
            bo_r = ptile([1, H], F32R, 'bo_r')
            nc.vector.tensor_copy(bo_r[:], bo_f[:])

            woT = ptile([128, H * HT], BF16, 'woT')

            qT = ptile([128, T], BF16, 'qT')
            kT = ptile([128, T], BF16, 'kT')
            v1 = ptile([128, NTT * 130], BF16, 'v1')

            def warm(n, src, width):
                """Keep the PE pstate ramp alive across a known idle window:
                back-to-back matmuls on resident data, result unused."""
                for _ in range(n):
                    wp = qpool.tile([128, 512], F32, tag='work', bufs=2,
                                    name='work')
                    nc.tensor.matmul(wp[0:128, 0:width],
                                     src[:, 0:128], src[:, 0:width],
                                     start=True, stop=True,
                                     skip_group_check=True)
            a2a_in0 = dpool.tile([NCHUNK, 64, TC], BF16)
            a2a_out0 = dpool.tile([NCHUNK, 64, TC], BF16)
            a2a_in1 = dpool.tile([NCHUNK, 64, TC], BF16)
            a2a_out1 = dpool.tile([NCHUNK, 64, TC], BF16)

            # v1 ones columns (col 64 of each 65-block pair), strided memset
            ones_dst = bass.AP(v1.tensor, v1.offset + 64,
                               [list(v1.ap[0]), [130, NTT], [65, 2]])
            nc.vector.memset(ones_dst, 1.0)

            # ---- L1: per-chunk QKV + head-0 attention ----
            if 'L' in phases:
                # spin the PE up while chunk 0 is still in flight (fp32
                # rank-1 matmuls: slow per-instruction, which is the point)
                warm(10, ones_f, 128)
                prev_fin = None
                for ch in range(NCHUNK):
                    xt = next_xt
                    if ch + 1 < NCHUNK:
                        next_xt = load_x(ch + 1)
                    for w_t, b_sb, dst in ((wqT, bq_sb, qT), (wkT, bk_sb, kT)):
                        ps = qpool.tile([128, 512], F32, tag='work', bufs=2,
                                        name='work')
                        for ht in range(HT):
                            nc.tensor.matmul(
                                ps[:], w_t[:, 128 * ht:128 * (ht + 1)],
                                xt[:, TC * ht:TC * (ht + 1)],
                                start=(ht == 0), stop=(ht == HT - 1))
                        nc.vector.tensor_scalar_add(
                            dst[:, TC * ch:TC * (ch + 1)], ps[:], b_sb[:, 0:1])
                    ps = qpool.tile([128, 512], F32, tag='work', bufs=2,
                                    name='work')
                    for ht in range(HT):
                        nc.tensor.matmul(
                            ps[:], wvT[:, 128 * ht:128 * (ht + 1)],
                            xt[:, TC * ht:TC * (ht + 1)],
                            start=(ht == 0), stop=(ht == HT - 1))
                    vt_sb = sc.tile([128, 512], BF16, tag='vt_sb', bufs=1,
                                    name='vt_sb')
                    nc.vector.tensor_scalar_add(vt_sb[:], ps[:], bv_sb[:, 0:1])
                    ps2 = qpool.tile([128, 1024], BF16, tag='work', bufs=2,
                                     name='work')
                    for tt in range(4):
                        nc.tensor.transpose(ps2[:, 128 * tt:128 * (tt + 1)],
                                            vt_sb[:, 128 * tt:128 * (tt + 1)],
                                            ident[:])
                    for tt in range(4):
                        base = 130 * (4 * ch + tt)
                        # [V_h0 | gap | V_h1]: one strided copy fills cols
                        # base..base+63 and base+65..base+128
                        dst = bass.AP(v1.tensor, v1.offset + base,
                                      [list(v1.ap[0]), [65, 2], [1, 64]])
                        nc.vector.tensor_copy(
                            dst,
                            ps2[:, 128 * tt:128 * (tt + 1)]
                            .rearrange('p (g c) -> p g c', g=2))
                    if prev_fin is not None:
                        prev_fin()
                    prev_fin = _attention(nc, sc, qpool, qT, kT, v1, ones_b,
                                          ut, a2a_in0, ch, 0,
                                          defer_finish=(ch + 1 < NCHUNK))
                if prev_fin is not None:
                    prev_fin()

                # woT only feeds E: load it behind all of L1's x traffic.
                nc.sync.dma_start(woT[:], wot_d[:])

                # ---- X0: AllToAll for head 0 (overlaps L2) ----
                nc.gpsimd.collective_compute(
                    'AllToAll', mybir.AluOpType.bypass,
                    replica_groups=[list(range(NC))],
                    ins=[a2a_in0[:]], outs=[a2a_out0[:]],
                )

            # ---- E prep: head-0 ctx loads overlap L2 (single DMA) ----
            ctxa = pp.tile([128, NC * TC], BF16, tag='ctxa', name='ctxa')
            if 'E' in phases and 'L' in phases:
                nc.sync.dma_start(
                    ctxa[0:64, :].rearrange('p (i t) -> p i t', i=NC),
                    a2a_out0[:, :, :].rearrange('i p t -> p i t'))

            # ---- L2: head-1 attention, expensive chunks first ----
            if 'L' in phases:
                _l2_pass(nc, sc, qpool, qT, kT, v1, ones_b, ut,
                         a2a_in1, (3, 0, 7, 4, 2, 6, 1, 5), 1)
                nc.gpsimd.collective_compute(
                    'AllToAll', mybir.AluOpType.bypass,
                    replica_groups=[list(range(NC))],
                    ins=[a2a_in1[:]], outs=[a2a_out1[:]],
                )

            # ---- E: output projection for my 512 tokens, split by head
            # half. The head-0 half (+bias) depends only on X0, so it runs
            # as real PE work inside the X1 collective window; after X1
            # only the head-1 half and a DVE add remain.
            if 'E' in phases:
                if 'L' in phases:
                    # head-1 ctx in two halves so E_h1 starts on the first
                    nc.sync.dma_start(
                        ctxa[64:128, 0:4 * TC].rearrange('p (i t) -> p i t',
                                                         i=4),
                        a2a_out1[0:4, :, :].rearrange('i p t -> p i t'))
                    nc.sync.dma_start(
                        ctxa[64:128, 4 * TC:].rearrange('p (i t) -> p i t',
                                                        i=4),
                        a2a_out1[4:8, :, :].rearrange('i p t -> p i t'))
                e0 = pp.tile([128, 8 * 512], F32, tag='e0', name='e0')
                for blk in range(8):
                    tt, oc = blk // 2, blk % 2
                    ps = qpool.tile([128, 512], F32, tag='work',
                                    bufs=2, name='work')
                    nc.tensor.matmul(ps[:], ones_r[0:1, 0:128],
                                     bo_r[0:1, 512 * oc:512 * (oc + 1)],
                                     start=True, stop=False)
                    for it in range(NC):
                        nc.tensor.matmul(
                            ps[:],
                            ctxa[0:64, TC * it + 128 * tt:TC * it + 128 * (tt + 1)],
                            woT[0:64, H * it + 512 * oc:H * it + 512 * (oc + 1)],
                            start=False, stop=(it == NC - 1))
                    nc.scalar.copy(e0[:, 512 * blk:512 * (blk + 1)], ps[:])
                if 'L' in phases:
                    # keep the PE hot for the rest of the X1 window; fine
                    # tail so overshoot past the ctxa arrival stays cheap
                    warm(52, woT, 512)
                    warm(75, woT, 128)
                for blk in range(8):
                    tt, oc = blk // 2, blk % 2
                    o_sb = sc.tile([128, 512], F32, tag='o_sb', bufs=3,
                                   name='o_sb')
                    ps = qpool.tile([128, 512], F32, tag='work',
                                    bufs=2, name='work')
                    for it in range(NC):
                        nc.tensor.matmul(
                            ps[:],
                            ctxa[64:128, TC * it + 128 * tt:TC * it + 128 * (tt + 1)],
                            woT[64:128, H * it + 512 * oc:H * it + 512 * (oc + 1)],
                            start=(it == 0), stop=(it == NC - 1))
                    nc.vector.tensor_add(o_sb[:], ps[:],
                                         e0[:, 512 * blk:512 * (blk + 1)])
                    nc.sync.dma_start(
                        out_d[128 * tt:128 * (tt + 1),
                              512 * oc:512 * (oc + 1)], o_sb[:])

    nc.compile()
    _cache[key] = nc
    return nc


def _wtile(w):
    """[H, 128] -> SBUF layout [128, HT*128]: [p, 128*ht+c] = w[128*ht+p, c]."""
    return np.ascontiguousarray(
        w.reshape(HT, 128, 128).transpose(1, 0, 2).reshape(128, H))


def kernel(hidden_states, Wq, bq, Wk, bk, Wv, bv, Wo, bo, **run_kwargs):
    import ml_dtypes
    bf16 = ml_dtypes.bfloat16
    nc = _build()
    hs = np.asarray(hidden_states, np.float32).reshape(T, H)
    # [ch, p, 512*ht+t] = x[512*ch+t, 128*ht+p]
    hst = np.ascontiguousarray(
        hs.astype(bf16).reshape(NCHUNK, TC, HT, 128).transpose(0, 3, 2, 1)
        .reshape(NCHUNK, 128, HT * TC))
    Wq, Wk, Wv, Wo = (np.asarray(w, np.float32) for w in (Wq, Wk, Wv, Wo))
    bq, bk, bv, bo = (np.asarray(b, np.float32) for b in (bq, bk, bv, bo))
    # [p, 1024*it+o] = Wo[o, 128*it+p]
    wot = np.ascontiguousarray(
        Wo.T.astype(bf16).reshape(HT, 128, H).transpose(1, 0, 2)
        .reshape(128, H * HT))
    in_maps = []
    for c in range(NC):
        r = slice(128 * c, 128 * (c + 1))
        in_maps.append({
            'hst': hst,
            'wqt': _wtile(Wq[r].T.astype(bf16)),
            'wkt': _wtile(Wk[r].T.astype(bf16)),
            'wvt': _wtile(Wv[r].T.astype(bf16)),
            'wot': wot,
            'bq': np.ascontiguousarray(bq[r].reshape(128, 1)),
            'bk': np.ascontiguousarray(bk[r].reshape(128, 1)),
            'bv': np.ascontiguousarray(bv[r].reshape(128, 1)),
            'bo': np.ascontiguousarray(bo.reshape(1, H)),
        })
    res = run_bass_kernel_spmd(nc, in_maps, core_ids=list(range(NC)), **run_kwargs)
    out = np.concatenate([res.results[c]['out'] for c in range(NC)], axis=0)
    kernel.last_results = res
    return out.reshape(B, S, H)
